# revision 18
# baseline (speedup 1.0000x reference)
import sys

import numpy as np

sys.path.insert(0, "/opt/trn_rl_repo")

B, S, D, K = 16384, 32, 64, 3
N_CORES = 8
P = 128  # partitions per tile
LN_EPS = 1e-6
ENT_EPS = 1e-10
SUM_EPS = 1e-6

_BUILD_CACHE = {}


def build_nc(nshard, apply_affine=False, use_custom_dve=False):
    """Build the single-core Bass/Tile graph for `nshard` samples.

    Layout: partition = sample (128 per tile), free = (z, s, d) with z = re/im.
    The LN of the written memory uses scale invariance:
        LN((1-eu)*mem + eu*gw) = LN(mem + f*gw),  f = eu/(1-eu)
    with the epsilon corrected by (1+f)^2 per (sample, slot).
    """
    from contextlib import ExitStack

    import concourse.bass as bass
    import concourse.mybir as mybir
    from concourse import bacc, tile

    dt = mybir.dt
    AF = mybir.ActivationFunctionType
    ALU = mybir.AluOpType
    AX = mybir.AxisListType

    ntiles = nshard // P
    ZSD = 2 * S * D  # 4096
    ZD = 2 * D  # 128

    nc = bacc.Bacc()

    mem_r = nc.declare_dram_parameter("mem_r", [nshard, S, D], dt.float32, isOutput=False)
    mem_i = nc.declare_dram_parameter("mem_i", [nshard, S, D], dt.float32, isOutput=False)
    gw_r = nc.declare_dram_parameter("gw_r", [nshard, D], dt.float32, isOutput=False)
    gw_i = nc.declare_dram_parameter("gw_i", [nshard, D], dt.float32, isOutput=False)
    wcat = nc.declare_dram_parameter("wcat", [ZD, S + 1], dt.float32, isOutput=False)
    bcat = nc.declare_dram_parameter("bcat", [P, S + 1], dt.float32, isOutput=False)
    ident = nc.declare_dram_parameter("ident", [P, P], dt.float32, isOutput=False)
    if apply_affine:
        grep = nc.declare_dram_parameter("grep", [P, ZD], dt.float32, isOutput=False)
        brep = nc.declare_dram_parameter("brep", [P, ZD], dt.float32, isOutput=False)

    next_r = nc.declare_dram_parameter("next_r", [nshard, S, D], dt.float32, isOutput=True)
    next_i = nc.declare_dram_parameter("next_i", [nshard, S, D], dt.float32, isOutput=True)
    read_r = nc.declare_dram_parameter("read_r", [nshard, D], dt.float32, isOutput=True)
    read_i = nc.declare_dram_parameter("read_i", [nshard, D], dt.float32, isOutput=True)
    ent = nc.declare_dram_parameter("ent", [P, 1], dt.float32, isOutput=True)

    with tile.TileContext(nc) as tc, ExitStack() as ctx:
        const = ctx.enter_context(tc.tile_pool(name="const", bufs=1))
        pmem = ctx.enter_context(tc.tile_pool(name="pmem", bufs=2))
        pmembf = ctx.enter_context(tc.tile_pool(name="pmembf", bufs=2))
        pscr = ctx.enter_context(tc.tile_pool(name="pscr", bufs=2))
        pscr2 = ctx.enter_context(tc.tile_pool(name="pscr2", bufs=2))
        pw = ctx.enter_context(tc.tile_pool(name="pw", bufs=2))
        pnext = ctx.enter_context(tc.tile_pool(name="pnext", bufs=2))
        pout = ctx.enter_context(tc.tile_pool(name="pout", bufs=2))
        pof = ctx.enter_context(tc.tile_pool(name="pof", bufs=2))
        psmall = ctx.enter_context(tc.tile_pool(name="psmall", bufs=2))
        ppsum = ctx.enter_context(
            tc.tile_pool(name="ppsum", bufs=2, space=bass.MemorySpace.PSUM)
        )

        # ---- constants (loaded once) ----
        wcat_t = const.tile([ZD, S + 1], dt.float32)
        nc.sync.dma_start(wcat_t[:], wcat[:])
        bcat_t = const.tile([P, S + 1], dt.float32)
        nc.sync.dma_start(bcat_t[:], bcat[:])
        ident_t = const.tile([P, P], dt.float32)
        nc.sync.dma_start(ident_t[:], ident[:])
        if apply_affine:
            grep_t = const.tile([P, ZD], dt.float32)
            nc.sync.dma_start(grep_t[:], grep[:])
            brep_t = const.tile([P, ZD], dt.float32)
            nc.sync.dma_start(brep_t[:], brep[:])
        ent_acc = const.tile([P, 2], dt.float32)
        nc.vector.memset(ent_acc[:], 0.0)
        entb = const.tile([P, 1], dt.float32)
        nc.vector.memset(entb[:], ENT_EPS)

        for i in range(ntiles):
            b0 = i * P
            bsl = slice(b0, b0 + P)

            # ---- DMA in ----
            memcat = pmem.tile([P, ZSD], dt.float32, tag="memcat")
            nc.sync.dma_start(memcat[:, 0 : S * D], mem_r[bsl])
            nc.sync.dma_start(memcat[:, S * D : ZSD], mem_i[bsl])
            gwcat = psmall.tile([P, ZD], dt.float32, tag="gwcat")
            nc.sync.dma_start(gwcat[:, 0:D], gw_r[bsl])
            nc.sync.dma_start(gwcat[:, D:ZD], gw_i[bsl])

            mem4 = memcat[:].rearrange("p (z s d) -> p z s d", z=2, s=S, d=D)
            gw_bz = (
                gwcat[:]
                .rearrange("p (z d) -> p z d", z=2)
                .unsqueeze(2)
                .broadcast_to((P, 2, S, D))
            )

            # ---- gate/address nets on PE ----
            # stage gw through ACT so the PE transpose carries only one
            # cross-engine wait (PE LW instructions have a tiny sync-wait cap)
            gws = psmall.tile([P, ZD], dt.float32, tag="gws")
            nc.scalar.copy(gws[:, 0:D], gwcat[:, 0:D])
            nc.scalar.copy(gws[:, D:ZD], gwcat[:, D:ZD])
            gwT_ps = ppsum.tile([P, P], dt.float32, tag="gwT")
            nc.tensor.transpose(gwT_ps[:], gws[:], ident_t[:])
            flatT = psmall.tile([P, P], dt.float32, tag="flatT")
            nc.scalar.copy(flatT[:], gwT_ps[:])
            lg_ps = ppsum.tile([P, S + 1], dt.float32, tag="lg")
            nc.tensor.matmul(lg_ps[:], flatT[:], wcat_t[:], start=True, stop=True)
            lg = psmall.tile([P, S + 1], dt.float32, tag="lg_sb")
            nc.vector.tensor_add(lg[:], lg_ps[:], bcat_t[:])

            # ---- similarity: sim = sum_d mem_r*gw_r + mem_i*gw_i ----
            # split r/i so each TT waits on only one mem DMA (sync-wait cap)
            t1 = pscr.tile([P, ZSD], dt.float32, tag="t1")
            t14 = t1[:].rearrange("p (z s d) -> p z s d", z=2, s=S, d=D)
            nc.vector.tensor_mul(t14[:, 0], mem4[:, 0], gw_bz[:, 0])
            nc.vector.tensor_mul(t14[:, 1], mem4[:, 1], gw_bz[:, 1])
            simzs = psmall.tile([P, 2 * S], dt.float32, tag="simzs")
            nc.vector.tensor_reduce(
                simzs[:],
                t1[:].rearrange("p (g d) -> p g d", g=2 * S, d=D),
                axis=AX.X,
                op=ALU.add,
            )
            simt = psmall.tile([P, S], dt.float32, tag="sim")
            nc.vector.tensor_add(simt[:], simzs[:, 0:S], simzs[:, S : 2 * S])

            # ---- attn softmax (no max-subtract; |sim| <~ 60 is exp-safe) ----
            es = psmall.tile([P, S], dt.float32, tag="es")
            nc.scalar.activation(es[:], simt[:], AF.Exp)
            zr = psmall.tile([P, 4], dt.float32, tag="zr")
            nc.vector.tensor_reduce(zr[:, 0:1], es[:], axis=AX.X, op=ALU.add)
            nc.vector.reciprocal(zr[:, 1:2], zr[:, 0:1])
            attn_bf = psmall.tile([P, S], dt.bfloat16, tag="attnbf")
            nc.vector.tensor_scalar(
                out=attn_bf[:], in0=es[:], scalar1=zr[:, 1:2], scalar2=None, op0=ALU.mult
            )

            # ---- write-gate softmax ww, gate, entropy, top-k ----
            eww = psmall.tile([P, S], dt.float32, tag="eww")
            nc.scalar.activation(eww[:], lg[:, 0:S], AF.Exp)
            zw = psmall.tile([P, 8], dt.float32, tag="zw")
            nc.vector.tensor_reduce(zw[:, 0:1], eww[:], axis=AX.X, op=ALU.add)
            nc.vector.reciprocal(zw[:, 1:2], zw[:, 0:1])
            ww = psmall.tile([P, S], dt.float32, tag="ww")
            nc.vector.tensor_scalar(
                out=ww[:], in0=eww[:], scalar1=zw[:, 1:2], scalar2=None, op0=ALU.mult
            )
            # gate = sigmoid(lg[:, S]) = 1/(1+exp(-x))
            nc.scalar.activation(zw[:, 2:3], lg[:, S : S + 1], AF.Exp, scale=-1.0)
            nc.vector.tensor_scalar(
                out=zw[:, 3:4], in0=zw[:, 2:3], scalar1=1.0, scalar2=None, op0=ALU.add
            )
            nc.vector.reciprocal(zw[:, 4:5], zw[:, 3:4])  # gate

            # entropy: ent += -sum ww*ln(ww+eps)
            lw = psmall.tile([P, S], dt.float32, tag="lw")
            nc.scalar.activation(lw[:], ww[:], AF.Ln, bias=entb[:])
            entj = psmall.tile([P, S], dt.float32, tag="entj")
            if use_custom_dve:
                nc.vector.tensor_tensor_reduce(
                    out=entj[:],
                    in0=ww[:],
                    in1=lw[:],
                    scale=-1.0,
                    scalar=ent_acc[:, i % 2 : i % 2 + 1],
                    op0=ALU.mult,
                    op1=ALU.add,
                    accum_out=ent_acc[:, (i + 1) % 2 : (i + 1) % 2 + 1],
                )
            else:
                nc.vector.tensor_mul(entj[:], ww[:], lw[:])
                nc.vector.tensor_reduce(zw[:, 7:8], entj[:], axis=AX.X, op=ALU.add)
                nc.vector.scalar_tensor_tensor(
                    out=ent_acc[:, (i + 1) % 2 : (i + 1) % 2 + 1],
                    in0=zw[:, 7:8],
                    scalar=-1.0,
                    in1=ent_acc[:, i % 2 : i % 2 + 1],
                    op0=ALU.mult,
                    op1=ALU.add,
                )

            # top-3 of ww -> normalized sparse weights -> eu -> f = eu/(1-eu)
            mask = psmall.tile([P, S], dt.float32, tag="mask")
            if use_custom_dve:
                m8 = psmall.tile([P, 8], dt.float32, tag="m8")
                nc.vector.max(m8[:], ww[:])
                nc.vector.tensor_scalar(
                    out=mask[:], in0=ww[:], scalar1=m8[:, K - 1 : K], scalar2=None, op0=ALU.is_ge
                )
            else:
                # iterative top-3 threshold via reduce_max + masking
                wtmp = psmall.tile([P, S], dt.float32, tag="wtmp")
                mk = psmall.tile([P, 4], dt.float32, tag="mk")
                nc.vector.tensor_reduce(mk[:, 0:1], ww[:], axis=AX.X, op=ALU.max)
                # wtmp = ww where ww < m1 else -1
                eqm = psmall.tile([P, S], dt.float32, tag="eqm")
                nc.vector.tensor_scalar(
                    out=eqm[:], in0=ww[:], scalar1=mk[:, 0:1], scalar2=None, op0=ALU.is_ge
                )
                nc.vector.scalar_tensor_tensor(
                    out=wtmp[:], in0=eqm[:], scalar=-2.0, in1=ww[:], op0=ALU.mult, op1=ALU.add
                )
                nc.vector.tensor_reduce(mk[:, 1:2], wtmp[:], axis=AX.X, op=ALU.max)
                nc.vector.tensor_scalar(
                    out=eqm[:], in0=wtmp[:], scalar1=mk[:, 1:2], scalar2=None, op0=ALU.is_ge
                )
                nc.vector.scalar_tensor_tensor(
                    out=wtmp[:], in0=eqm[:], scalar=-2.0, in1=wtmp[:], op0=ALU.mult, op1=ALU.add
                )
                nc.vector.tensor_reduce(mk[:, 2:3], wtmp[:], axis=AX.X, op=ALU.max)
                nc.vector.tensor_scalar(
                    out=mask[:], in0=ww[:], scalar1=mk[:, 2:3], scalar2=None, op0=ALU.is_ge
                )
            spv = psmall.tile([P, S], dt.float32, tag="spv")
            if use_custom_dve:
                nc.vector.tensor_tensor_reduce(
                    out=spv[:],
                    in0=ww[:],
                    in1=mask[:],
                    scale=1.0,
                    scalar=float(SUM_EPS),
                    op0=ALU.mult,
                    op1=ALU.add,
                    accum_out=zw[:, 5:6],
                )
            else:
                nc.vector.tensor_mul(spv[:], ww[:], mask[:])
                nc.vector.tensor_reduce(zw[:, 6:7], spv[:], axis=AX.X, op=ALU.add)
                nc.vector.tensor_scalar(
                    out=zw[:, 5:6], in0=zw[:, 6:7], scalar1=float(SUM_EPS), scalar2=None, op0=ALU.add
                )
            nc.vector.reciprocal(zw[:, 6:7], zw[:, 5:6])
            eu = psmall.tile([P, S], dt.float32, tag="eu")
            nc.vector.tensor_scalar(
                out=eu[:],
                in0=spv[:],
                scalar1=zw[:, 6:7],
                scalar2=zw[:, 4:5],
                op0=ALU.mult,
                op1=ALU.mult,
            )
            ome = psmall.tile([P, S], dt.float32, tag="ome")
            nc.vector.tensor_scalar(
                out=ome[:], in0=eu[:], scalar1=-1.0, scalar2=1.0, op0=ALU.mult, op1=ALU.add
            )
            rome = psmall.tile([P, S], dt.float32, tag="rome")
            nc.vector.reciprocal(rome[:], ome[:])
            f_t = psmall.tile([P, S], dt.float32, tag="f")
            nc.vector.tensor_mul(f_t[:], eu[:], rome[:])

            # ---- w = f * gw  (per-slot ACT copy with AP scale), bf16 ----
            # gw is staged through an ACT-local bf16 copy so the 32 w-loop
            # activations only carry the f_t cross-engine wait (walrus caps
            # sync-wait commands per instruction).
            # each half-copy waits on only one DMA sem (ACT sync-wait cap)
            mem_bf = pmembf.tile([P, ZSD], dt.bfloat16, tag="membf")
            nc.scalar.copy(mem_bf[:, 0 : S * D], memcat[:, 0 : S * D])
            nc.scalar.copy(mem_bf[:, S * D : ZSD], memcat[:, S * D : ZSD])
            gw_bf = psmall.tile([P, ZD], dt.bfloat16, tag="gwbf")
            nc.scalar.copy(gw_bf[:, 0:D], gwcat[:, 0:D])
            nc.scalar.copy(gw_bf[:, D:ZD], gwcat[:, D:ZD])
            w_bf = pw.tile([P, ZSD], dt.bfloat16, tag="w")
            w4 = w_bf[:].rearrange("p (z s d) -> p z s d", z=2, s=S, d=D)
            gw3 = gw_bf[:].rearrange("p (z d) -> p z d", z=2)
            for s in range(S):
                nc.scalar.activation(
                    w4[:, :, s, :], gw3, AF.Copy, scale=f_t[:, s : s + 1]
                )
            next_bf = pnext.tile([P, ZSD], dt.bfloat16, tag="next")
            nc.vector.tensor_add(next_bf[:], mem_bf[:], w_bf[:])

            # LN stats of next': mean and E[x^2] via square + segmented reduces
            sq_bf = pscr2.tile([P, ZSD], dt.bfloat16, tag="sqbf")
            nc.scalar.square(sq_bf[:], next_bf[:])
            sn = psmall.tile([P, 2 * S], dt.float32, tag="sn")
            nc.vector.tensor_reduce(
                sn[:],
                next_bf[:].rearrange("p (g d) -> p g d", g=2 * S, d=D),
                axis=AX.X,
                op=ALU.add,
            )
            sq = psmall.tile([P, 2 * S], dt.float32, tag="sqs")
            nc.vector.tensor_reduce(
                sq[:],
                sq_bf[:].rearrange("p (g d) -> p g d", g=2 * S, d=D),
                axis=AX.X,
                op=ALU.add,
            )
            mmean = psmall.tile([P, 2 * S], dt.float32, tag="mmean")
            nc.vector.tensor_scalar(
                out=mmean[:], in0=sn[:], scalar1=1.0 / D, scalar2=None, op0=ALU.mult
            )
            msq = psmall.tile([P, 2 * S], dt.float32, tag="msq")
            nc.scalar.square(msq[:], mmean[:])
            varr = psmall.tile([P, 2 * S], dt.float32, tag="varr")
            nc.vector.scalar_tensor_tensor(
                out=varr[:], in0=sq[:], scalar=1.0 / D, in1=msq[:], op0=ALU.mult, op1=ALU.subtract
            )
            # eps correction: varf = varr + LN_EPS*(1+f)^2
            opf2 = psmall.tile([P, S], dt.float32, tag="opf2")
            nc.scalar.activation(opf2[:], f_t[:], AF.Square, bias=1.0)
            varf = psmall.tile([P, 2 * S], dt.float32, tag="varf")
            nc.vector.scalar_tensor_tensor(
                out=varf[:].rearrange("p (z s) -> p z s", z=2),
                in0=opf2[:].unsqueeze(1).broadcast_to((P, 2, S)),
                scalar=LN_EPS,
                in1=varr[:].rearrange("p (z s) -> p z s", z=2),
                op0=ALU.mult,
                op1=ALU.add,
            )
            # rs = 1/sqrt(varf) via exp(-0.5*ln(varf))
            lv = psmall.tile([P, 2 * S], dt.float32, tag="lv")
            nc.scalar.activation(lv[:], varf[:], AF.Ln)
            rs = psmall.tile([P, 2 * S], dt.float32, tag="rs")
            nc.scalar.activation(rs[:], lv[:], AF.Exp, scale=-0.5)
            cc = psmall.tile([P, 2 * S], dt.float32, tag="cc")
            nc.vector.scalar_tensor_tensor(
                out=cc[:], in0=mmean[:], scalar=-1.0, in1=rs[:], op0=ALU.mult, op1=ALU.mult
            )

            # ---- apply LN: out = next'*rs + cc, per (z,s) group ----
            out_bf = pout.tile([P, ZSD], dt.bfloat16, tag="out")
            for g in range(2 * S):
                nc.vector.tensor_scalar(
                    out=out_bf[:, g * D : (g + 1) * D],
                    in0=next_bf[:, g * D : (g + 1) * D],
                    scalar1=rs[:, g : g + 1],
                    scalar2=cc[:, g : g + 1],
                    op0=ALU.mult,
                    op1=ALU.add,
                )
            out_f32 = pof.tile([P, ZSD], dt.float32, tag="outf")
            nc.scalar.copy(out_f32[:, 0 : S * D], out_bf[:, 0 : S * D])
            nc.scalar.copy(out_f32[:, S * D : ZSD], out_bf[:, S * D : ZSD])
            if apply_affine:
                o2 = pscr2.tile([P, ZSD], dt.bfloat16, tag="o2")
                g_b = (
                    grep_t[:]
                    .rearrange("p (z d) -> p z d", z=2)
                    .unsqueeze(2)
                    .broadcast_to((P, 2, S, D))
                )
                b_b = (
                    brep_t[:]
                    .rearrange("p (z d) -> p z d", z=2)
                    .unsqueeze(2)
                    .broadcast_to((P, 2, S, D))
                )
                o4 = out_bf[:].rearrange("p (z s d) -> p z s d", z=2, s=S, d=D)
                nc.vector.tensor_mul(o2[:].rearrange("p (z s d) -> p z s d", z=2, s=S, d=D), o4, g_b)
                nc.vector.tensor_add(
                    out_bf[:].rearrange("p (z s d) -> p z s d", z=2, s=S, d=D),
                    o2[:].rearrange("p (z s d) -> p z s d", z=2, s=S, d=D),
                    b_b,
                )

            # ---- read = sum_s attn*mem ----
            t2 = pscr2.tile([P, ZSD], dt.bfloat16, tag="t2")
            attn_b = (
                attn_bf[:]
                .unsqueeze(1)
                .unsqueeze(3)
                .broadcast_to((P, 2, S, D))
            )
            mem4bf = mem_bf[:].rearrange("p (z s d) -> p z s d", z=2, s=S, d=D)
            nc.vector.tensor_mul(
                t2[:].rearrange("p (z s d) -> p z s d", z=2, s=S, d=D), mem4bf, attn_b
            )
            readc = psmall.tile([P, 2, D], dt.float32, tag="readc")
            nc.vector.tensor_reduce(
                readc[:],
                t2[:].rearrange("p (z s d) -> p z d s", z=2, s=S, d=D),
                axis=AX.X,
                op=ALU.add,
            )

            # ---- DMA out ----
            nc.sync.dma_start(read_r[bsl], readc[:, 0, :])
            nc.sync.dma_start(read_i[bsl], readc[:, 1, :])
            nc.sync.dma_start(next_r[bsl], out_f32[:, 0 : S * D])
            nc.sync.dma_start(next_i[bsl], out_f32[:, S * D : ZSD])

        nc.sync.dma_start(ent[:], ent_acc[:, ntiles % 2 : ntiles % 2 + 1])

    nc.finalize()
    return nc


def _prep_shards(gw_r, gw_i, mem_r, mem_i, Wg, bg, Wa, ba):
    nshard = gw_r.shape[0] // N_CORES
    wcat = np.concatenate([Wa, Wg], axis=1).astype(np.float32)  # [2D, S+1]
    bcat = np.broadcast_to(
        np.concatenate([ba, bg]).astype(np.float32)[None, :], (P, S + 1)
    ).copy()
    ident = np.eye(P, dtype=np.float32)
    in_maps = []
    for c in range(N_CORES):
        sl = slice(c * nshard, (c + 1) * nshard)
        in_maps.append(
            {
                "mem_r": np.ascontiguousarray(mem_r[sl]),
                "mem_i": np.ascontiguousarray(mem_i[sl]),
                "gw_r": np.ascontiguousarray(gw_r[sl]),
                "gw_i": np.ascontiguousarray(gw_i[sl]),
                "wcat": wcat,
                "bcat": bcat,
                "ident": ident,
            }
        )
    return in_maps


def kernel(gw_r, gw_i, mem_r, mem_i, Wg, bg, Wa, ba, g_r, b_r, g_i, b_i, _trace=False):
    gw_r = np.asarray(gw_r, dtype=np.float32)
    gw_i = np.asarray(gw_i, dtype=np.float32)
    mem_r = np.asarray(mem_r, dtype=np.float32)
    mem_i = np.asarray(mem_i, dtype=np.float32)
    Wg = np.asarray(Wg, dtype=np.float32)
    bg = np.asarray(bg, dtype=np.float32)
    Wa = np.asarray(Wa, dtype=np.float32)
    ba = np.asarray(ba, dtype=np.float32)
    g_r = np.asarray(g_r, dtype=np.float32)
    b_r = np.asarray(b_r, dtype=np.float32)
    g_i = np.asarray(g_i, dtype=np.float32)
    b_i = np.asarray(b_i, dtype=np.float32)

    from concourse.bass_utils import run_bass_kernel_spmd

    nb = gw_r.shape[0]
    nshard = nb // N_CORES
    apply_affine = not (
        np.all(g_r == 1.0) and np.all(g_i == 1.0) and np.all(b_r == 0.0) and np.all(b_i == 0.0)
    )
    key = (nshard, apply_affine)
    if key not in _BUILD_CACHE:
        _BUILD_CACHE[key] = build_nc(nshard, apply_affine)
    nc = _BUILD_CACHE[key]

    in_maps = _prep_shards(gw_r, gw_i, mem_r, mem_i, Wg, bg, Wa, ba)
    if apply_affine:
        grep = np.broadcast_to(
            np.concatenate([g_r, g_i]).astype(np.float32)[None, :], (P, 2 * D)
        ).copy()
        brep = np.broadcast_to(
            np.concatenate([b_r, b_i]).astype(np.float32)[None, :], (P, 2 * D)
        ).copy()
        for m in in_maps:
            m["grep"] = grep
            m["brep"] = brep

    res = run_bass_kernel_spmd(nc, in_maps, core_ids=list(range(N_CORES)), trace=_trace)
    outs = res.results

    next_r = np.concatenate([outs[c]["next_r"] for c in range(N_CORES)], axis=0)
    next_i = np.concatenate([outs[c]["next_i"] for c in range(N_CORES)], axis=0)
    read_r = np.concatenate([outs[c]["read_r"] for c in range(N_CORES)], axis=0)
    read_i = np.concatenate([outs[c]["read_i"] for c in range(N_CORES)], axis=0)
    ent_total = sum(float(outs[c]["ent"].sum()) for c in range(N_CORES))
    slot_entropy = np.float32(ent_total / nb)

    if _trace:
        kernel._last_result = res
    return (read_r, read_i, next_r, next_i, slot_entropy)


# revision 30
# speedup vs baseline: 3.6822x; 3.6822x over previous
import sys

import numpy as np

sys.path.insert(0, "/opt/trn_rl_repo")

B, S, D, K = 16384, 32, 64, 3
N_CORES = 8
P = 128  # partitions per tile
LN_EPS = 1e-6
ENT_EPS = 1e-10
SUM_EPS = 1e-6

_BUILD_CACHE = {}


def build_nc(nshard, apply_affine=False, use_custom_dve=False, skip=(), repeat=1):
    """Build the single-core Bass/Tile graph for `nshard` samples.

    Layout: partition = sample (128 per tile), free = (z, s, d) with z = re/im.
    The LN of the written memory uses scale invariance:
        LN((1-eu)*mem + eu*gw) = LN(mem + f*gw),  f = eu/(1-eu)
    with the epsilon corrected by (1+f)^2 per (sample, slot).

    `skip` is a set of stage names ("gate","sim","w","stats","apply","read")
    used for performance bisection; skipped stages are replaced by cheap
    constants so the rest of the pipeline still runs.
    """
    from contextlib import ExitStack

    import concourse.bass as bass
    import concourse.mybir as mybir
    from concourse import bacc, tile

    dt = mybir.dt
    AF = mybir.ActivationFunctionType
    ALU = mybir.AluOpType
    AX = mybir.AxisListType

    sk = set(skip)
    ntiles = nshard // P
    ZSD = 2 * S * D  # 4096
    ZD = 2 * D  # 128

    nc = bacc.Bacc()

    mem_r = nc.declare_dram_parameter("mem_r", [nshard, S, D], dt.float32, isOutput=False)
    mem_i = nc.declare_dram_parameter("mem_i", [nshard, S, D], dt.float32, isOutput=False)
    gw_r = nc.declare_dram_parameter("gw_r", [nshard, D], dt.float32, isOutput=False)
    gw_i = nc.declare_dram_parameter("gw_i", [nshard, D], dt.float32, isOutput=False)
    wcat = nc.declare_dram_parameter("wcat", [ZD, S + 1], dt.float32, isOutput=False)
    bcat = nc.declare_dram_parameter("bcat", [P, S + 1], dt.float32, isOutput=False)
    ident = nc.declare_dram_parameter("ident", [P, P], dt.float32, isOutput=False)
    if apply_affine:
        grep = nc.declare_dram_parameter("grep", [P, ZD], dt.float32, isOutput=False)
        brep = nc.declare_dram_parameter("brep", [P, ZD], dt.float32, isOutput=False)

    next_r = nc.declare_dram_parameter("next_r", [nshard, S, D], dt.float32, isOutput=True)
    next_i = nc.declare_dram_parameter("next_i", [nshard, S, D], dt.float32, isOutput=True)
    read_r = nc.declare_dram_parameter("read_r", [nshard, D], dt.float32, isOutput=True)
    read_i = nc.declare_dram_parameter("read_i", [nshard, D], dt.float32, isOutput=True)
    ent = nc.declare_dram_parameter("ent", [P, 1], dt.float32, isOutput=True)

    with tile.TileContext(nc) as tc, ExitStack() as ctx:
        const = ctx.enter_context(tc.tile_pool(name="const", bufs=1))
        pmem = ctx.enter_context(tc.tile_pool(name="pmem", bufs=2))
        pmembf = ctx.enter_context(tc.tile_pool(name="pmembf", bufs=2))
        pscr2 = ctx.enter_context(tc.tile_pool(name="pscr2", bufs=3))
        pw = ctx.enter_context(tc.tile_pool(name="pw", bufs=2))
        pnext = ctx.enter_context(tc.tile_pool(name="pnext", bufs=2))
        pof = ctx.enter_context(tc.tile_pool(name="pof", bufs=3))
        psmall = ctx.enter_context(tc.tile_pool(name="psmall", bufs=3))
        ppsum = ctx.enter_context(
            tc.tile_pool(name="ppsum", bufs=2, space=bass.MemorySpace.PSUM)
        )

        # ---- constants (loaded once) ----
        wcat_t = const.tile([ZD, S + 1], dt.float32)
        nc.sync.dma_start(wcat_t[:], wcat[:])
        bcat_t = const.tile([P, S + 1], dt.float32)
        nc.sync.dma_start(bcat_t[:], bcat[:])
        ident_t = const.tile([P, P], dt.float32)
        nc.sync.dma_start(ident_t[:], ident[:])
        if apply_affine:
            grep_t = const.tile([P, ZD], dt.float32)
            nc.sync.dma_start(grep_t[:], grep[:])
            brep_t = const.tile([P, ZD], dt.float32)
            nc.sync.dma_start(brep_t[:], brep[:])
        ent_acc = const.tile([P, 2], dt.float32)
        nc.vector.memset(ent_acc[:], 0.0)
        entb = const.tile([P, 1], dt.float32)
        nc.vector.memset(entb[:], ENT_EPS)

        for i in range(ntiles * repeat):
            i = i % ntiles
            b0 = i * P
            bsl = slice(b0, b0 + P)

            # ---- DMA in ----
            memcat = pmem.tile([P, ZSD], dt.float32, tag="memcat")
            nc.sync.dma_start(memcat[:, 0 : S * D], mem_r[bsl])
            nc.sync.dma_start(memcat[:, S * D : ZSD], mem_i[bsl])
            gwcat = psmall.tile([P, ZD], dt.float32, tag="gwcat")
            nc.sync.dma_start(gwcat[:, 0:D], gw_r[bsl])
            nc.sync.dma_start(gwcat[:, D:ZD], gw_i[bsl])

            mem4 = memcat[:].rearrange("p (z s d) -> p z s d", z=2, s=S, d=D)
            gw_bz = (
                gwcat[:]
                .rearrange("p (z d) -> p z d", z=2)
                .unsqueeze(2)
                .broadcast_to((P, 2, S, D))
            )

            # bf16 staging copies on ACT (each waits on only one DMA sem)
            mem_bf = pmembf.tile([P, ZSD], dt.bfloat16, tag="membf")
            nc.scalar.copy(mem_bf[:, 0 : S * D], memcat[:, 0 : S * D])
            nc.scalar.copy(mem_bf[:, S * D : ZSD], memcat[:, S * D : ZSD])

            # ---- gate/address nets (PE) + ww softmax + top-k -> f ----
            f_t = psmall.tile([P, S], dt.float32, tag="f")
            if "gate" not in sk:
                gws = psmall.tile([P, ZD], dt.float32, tag="gws")
                nc.scalar.copy(gws[:, 0:D], gwcat[:, 0:D])
                nc.scalar.copy(gws[:, D:ZD], gwcat[:, D:ZD])
                gwT_ps = ppsum.tile([P, P], dt.float32, tag="gwT")
                nc.tensor.transpose(gwT_ps[:], gws[:], ident_t[:])
                flatT = psmall.tile([P, P], dt.float32, tag="flatT")
                nc.scalar.copy(flatT[:], gwT_ps[:])
                lg_ps = ppsum.tile([P, S + 1], dt.float32, tag="lg")
                nc.tensor.matmul(lg_ps[:], flatT[:], wcat_t[:], start=True, stop=True)
                lg = psmall.tile([P, S + 1], dt.float32, tag="lg_sb")
                nc.vector.tensor_add(lg[:], lg_ps[:], bcat_t[:])

                eww = psmall.tile([P, S], dt.float32, tag="eww")
                nc.scalar.activation(eww[:], lg[:, 0:S], AF.Exp)
                zw = psmall.tile([P, 8], dt.float32, tag="zw")
                nc.vector.tensor_reduce(zw[:, 0:1], eww[:], axis=AX.X, op=ALU.add)
                nc.vector.reciprocal(zw[:, 1:2], zw[:, 0:1])
                ww = psmall.tile([P, S], dt.float32, tag="ww")
                nc.vector.tensor_scalar(
                    out=ww[:], in0=eww[:], scalar1=zw[:, 1:2], scalar2=None, op0=ALU.mult
                )
                # gate = sigmoid(lg[:, S]) = 1/(1+exp(-x))
                nc.scalar.activation(zw[:, 2:3], lg[:, S : S + 1], AF.Exp, scale=-1.0)
                nc.vector.tensor_scalar(
                    out=zw[:, 3:4], in0=zw[:, 2:3], scalar1=1.0, scalar2=None, op0=ALU.add
                )
                nc.vector.reciprocal(zw[:, 4:5], zw[:, 3:4])  # gate

                # entropy: ent += -sum ww*ln(ww+eps)
                lw = psmall.tile([P, S], dt.float32, tag="lw")
                nc.scalar.activation(lw[:], ww[:], AF.Ln, bias=entb[:])
                entj = psmall.tile([P, S], dt.float32, tag="entj")
                if use_custom_dve:
                    nc.vector.tensor_tensor_reduce(
                        out=entj[:],
                        in0=ww[:],
                        in1=lw[:],
                        scale=-1.0,
                        scalar=ent_acc[:, i % 2 : i % 2 + 1],
                        op0=ALU.mult,
                        op1=ALU.add,
                        accum_out=ent_acc[:, (i + 1) % 2 : (i + 1) % 2 + 1],
                    )
                else:
                    nc.vector.tensor_mul(entj[:], ww[:], lw[:])
                    nc.vector.tensor_reduce(zw[:, 7:8], entj[:], axis=AX.X, op=ALU.add)
                    nc.vector.scalar_tensor_tensor(
                        out=ent_acc[:, (i + 1) % 2 : (i + 1) % 2 + 1],
                        in0=zw[:, 7:8],
                        scalar=-1.0,
                        in1=ent_acc[:, i % 2 : i % 2 + 1],
                        op0=ALU.mult,
                        op1=ALU.add,
                    )

                # top-3 threshold -> mask -> sparse -> eu -> f = eu/(1-eu)
                mask = psmall.tile([P, S], dt.float32, tag="mask")
                if use_custom_dve:
                    m8 = psmall.tile([P, 8], dt.float32, tag="m8")
                    nc.vector.max(m8[:], ww[:])
                    nc.vector.tensor_scalar(
                        out=mask[:], in0=ww[:], scalar1=m8[:, K - 1 : K],
                        scalar2=None, op0=ALU.is_ge,
                    )
                else:
                    wtmp = psmall.tile([P, S], dt.float32, tag="wtmp")
                    mk = psmall.tile([P, 4], dt.float32, tag="mk")
                    eqm = psmall.tile([P, S], dt.float32, tag="eqm")
                    nc.vector.tensor_reduce(mk[:, 0:1], ww[:], axis=AX.X, op=ALU.max)
                    nc.vector.tensor_scalar(
                        out=eqm[:], in0=ww[:], scalar1=mk[:, 0:1], scalar2=None, op0=ALU.is_ge
                    )
                    nc.vector.scalar_tensor_tensor(
                        out=wtmp[:], in0=eqm[:], scalar=-2.0, in1=ww[:], op0=ALU.mult, op1=ALU.add
                    )
                    nc.vector.tensor_reduce(mk[:, 1:2], wtmp[:], axis=AX.X, op=ALU.max)
                    nc.vector.tensor_scalar(
                        out=eqm[:], in0=wtmp[:], scalar1=mk[:, 1:2], scalar2=None, op0=ALU.is_ge
                    )
                    nc.vector.scalar_tensor_tensor(
                        out=wtmp[:], in0=eqm[:], scalar=-2.0, in1=wtmp[:], op0=ALU.mult, op1=ALU.add
                    )
                    nc.vector.tensor_reduce(mk[:, 2:3], wtmp[:], axis=AX.X, op=ALU.max)
                    nc.vector.tensor_scalar(
                        out=mask[:], in0=ww[:], scalar1=mk[:, 2:3], scalar2=None, op0=ALU.is_ge
                    )
                spv = psmall.tile([P, S], dt.float32, tag="spv")
                if use_custom_dve:
                    nc.vector.tensor_tensor_reduce(
                        out=spv[:],
                        in0=ww[:],
                        in1=mask[:],
                        scale=1.0,
                        scalar=float(SUM_EPS),
                        op0=ALU.mult,
                        op1=ALU.add,
                        accum_out=zw[:, 5:6],
                    )
                else:
                    nc.vector.tensor_mul(spv[:], ww[:], mask[:])
                    nc.vector.tensor_reduce(zw[:, 6:7], spv[:], axis=AX.X, op=ALU.add)
                    nc.vector.tensor_scalar(
                        out=zw[:, 5:6], in0=zw[:, 6:7], scalar1=float(SUM_EPS),
                        scalar2=None, op0=ALU.add,
                    )
                nc.vector.reciprocal(zw[:, 6:7], zw[:, 5:6])
                eu = psmall.tile([P, S], dt.float32, tag="eu")
                nc.vector.tensor_scalar(
                    out=eu[:],
                    in0=spv[:],
                    scalar1=zw[:, 6:7],
                    scalar2=zw[:, 4:5],
                    op0=ALU.mult,
                    op1=ALU.mult,
                )
                ome = psmall.tile([P, S], dt.float32, tag="ome")
                nc.vector.tensor_scalar(
                    out=ome[:], in0=eu[:], scalar1=-1.0, scalar2=1.0, op0=ALU.mult, op1=ALU.add
                )
                rome = psmall.tile([P, S], dt.float32, tag="rome")
                nc.vector.reciprocal(rome[:], ome[:])
                nc.vector.tensor_mul(f_t[:], eu[:], rome[:])
            else:
                nc.vector.memset(f_t[:], 0.1)

            # ---- similarity + attn softmax ----
            attn_bf = psmall.tile([P, S], dt.bfloat16, tag="attnbf")
            if "sim" not in sk:
                # split r/i so each TT waits on only one mem DMA (sync-wait cap)
                t1 = pof.tile([P, ZSD], dt.float32, tag="outf")
                t14 = t1[:].rearrange("p (z s d) -> p z s d", z=2, s=S, d=D)
                nc.vector.tensor_mul(t14[:, 0], mem4[:, 0], gw_bz[:, 0])
                nc.vector.tensor_mul(t14[:, 1], mem4[:, 1], gw_bz[:, 1])
                simzs = psmall.tile([P, 2 * S], dt.float32, tag="simzs")
                nc.vector.tensor_reduce(
                    simzs[:],
                    t1[:].rearrange("p (g d) -> p g d", g=2 * S, d=D),
                    axis=AX.X,
                    op=ALU.add,
                )
                simt = psmall.tile([P, S], dt.float32, tag="sim")
                nc.vector.tensor_add(simt[:], simzs[:, 0:S], simzs[:, S : 2 * S])
                # softmax without max-subtract; |sim| <~ 60 is exp-safe
                es = psmall.tile([P, S], dt.float32, tag="es")
                nc.scalar.activation(es[:], simt[:], AF.Exp)
                zr = psmall.tile([P, 4], dt.float32, tag="zr")
                nc.vector.tensor_reduce(zr[:, 0:1], es[:], axis=AX.X, op=ALU.add)
                nc.vector.reciprocal(zr[:, 1:2], zr[:, 0:1])
                nc.vector.tensor_scalar(
                    out=attn_bf[:], in0=es[:], scalar1=zr[:, 1:2], scalar2=None, op0=ALU.mult
                )
            else:
                nc.vector.memset(attn_bf[:], 1.0 / S)

            # ---- w = f*gw; next' = mem + w ----
            if "w" not in sk:
                gw_bf = psmall.tile([P, ZD], dt.bfloat16, tag="gwbf")
                nc.scalar.copy(gw_bf[:, 0:D], gwcat[:, 0:D])
                nc.scalar.copy(gw_bf[:, D:ZD], gwcat[:, D:ZD])
                w_bf = pw.tile([P, ZSD], dt.bfloat16, tag="w")
                w4 = w_bf[:].rearrange("p (z s d) -> p z s d", z=2, s=S, d=D)
                gw3 = gw_bf[:].rearrange("p (z d) -> p z d", z=2)
                for s in range(S):
                    nc.scalar.activation(
                        w4[:, :, s, :], gw3, AF.Copy, scale=f_t[:, s : s + 1]
                    )
                next_bf = pnext.tile([P, ZSD], dt.bfloat16, tag="next")
                nc.vector.tensor_add(next_bf[:], mem_bf[:], w_bf[:])
            else:
                next_bf = mem_bf

            # ---- LN stats of next' ----
            rs = psmall.tile([P, 2 * S], dt.float32, tag="rs")
            cc = psmall.tile([P, 2 * S], dt.float32, tag="cc")
            if "stats" not in sk:
                sq_bf = pscr2.tile([P, ZSD], dt.bfloat16, tag="sqbf")
                nc.scalar.square(sq_bf[:], next_bf[:])
                sn = psmall.tile([P, 2 * S], dt.float32, tag="sn")
                snt = pscr2.tile([P, 2 * S, D // 2], dt.bfloat16, tag="t2")
                nx4 = next_bf[:].rearrange("p (g d) -> p g d", g=2 * S, d=D)
                nc.vector.tensor_add(snt[:], nx4[:, :, 0 : D // 2], nx4[:, :, D // 2 : D])
                hd = D // 2
                while hd > 2:
                    hd //= 2
                    nc.vector.tensor_add(
                        snt[:, :, 0:hd], snt[:, :, 0:hd], snt[:, :, hd : 2 * hd]
                    )
                nc.vector.tensor_add(sn[:], snt[:, :, 0], snt[:, :, 1])
                sq = psmall.tile([P, 2 * S], dt.float32, tag="sqs")
                sq4 = sq_bf[:].rearrange("p (g d) -> p g d", g=2 * S, d=D)
                hd = D
                while hd > 2:
                    hd //= 2
                    nc.vector.tensor_add(
                        sq4[:, :, 0:hd], sq4[:, :, 0:hd], sq4[:, :, hd : 2 * hd]
                    )
                nc.vector.tensor_add(sq[:], sq4[:, :, 0], sq4[:, :, 1])
                mmean = psmall.tile([P, 2 * S], dt.float32, tag="mmean")
                nc.vector.tensor_scalar(
                    out=mmean[:], in0=sn[:], scalar1=1.0 / D, scalar2=None, op0=ALU.mult
                )
                msq = psmall.tile([P, 2 * S], dt.float32, tag="msq")
                nc.scalar.square(msq[:], mmean[:])
                varr = psmall.tile([P, 2 * S], dt.float32, tag="varr")
                nc.vector.scalar_tensor_tensor(
                    out=varr[:], in0=sq[:], scalar=1.0 / D, in1=msq[:],
                    op0=ALU.mult, op1=ALU.subtract,
                )
                # eps correction: varf = varr + LN_EPS*(1+f)^2
                opf2 = psmall.tile([P, S], dt.float32, tag="opf2")
                nc.scalar.activation(opf2[:], f_t[:], AF.Square, bias=1.0)
                varf = psmall.tile([P, 2 * S], dt.float32, tag="varf")
                nc.vector.scalar_tensor_tensor(
                    out=varf[:].rearrange("p (z s) -> p z s", z=2),
                    in0=opf2[:].unsqueeze(1).broadcast_to((P, 2, S)),
                    scalar=LN_EPS,
                    in1=varr[:].rearrange("p (z s) -> p z s", z=2),
                    op0=ALU.mult,
                    op1=ALU.add,
                )
                # rs = 1/sqrt(varf) via exp(-0.5*ln(varf))
                lv = psmall.tile([P, 2 * S], dt.float32, tag="lv")
                nc.scalar.activation(lv[:], varf[:], AF.Ln)
                nc.scalar.activation(rs[:], lv[:], AF.Exp, scale=-0.5)
                nc.vector.scalar_tensor_tensor(
                    out=cc[:], in0=mmean[:], scalar=-1.0, in1=rs[:], op0=ALU.mult, op1=ALU.mult
                )
            else:
                nc.vector.memset(rs[:], 1.0)
                nc.vector.memset(cc[:], 0.0)

            # ---- apply LN: out = next'*rs + cc, per (z,s) group ----
            out_f32 = pof.tile([P, ZSD], dt.float32, tag="outf")
            if "apply" not in sk:
                out_bf = pw.tile([P, ZSD], dt.bfloat16, tag="w")
                for g in range(2 * S):
                    nc.vector.tensor_scalar(
                        out=out_bf[:, g * D : (g + 1) * D],
                        in0=next_bf[:, g * D : (g + 1) * D],
                        scalar1=rs[:, g : g + 1],
                        scalar2=cc[:, g : g + 1],
                        op0=ALU.mult,
                        op1=ALU.add,
                    )
                if apply_affine:
                    o2 = pscr2.tile([P, ZSD], dt.bfloat16, tag="o2")
                    g_b = (
                        grep_t[:]
                        .rearrange("p (z d) -> p z d", z=2)
                        .unsqueeze(2)
                        .broadcast_to((P, 2, S, D))
                    )
                    b_b = (
                        brep_t[:]
                        .rearrange("p (z d) -> p z d", z=2)
                        .unsqueeze(2)
                        .broadcast_to((P, 2, S, D))
                    )
                    o4 = out_bf[:].rearrange("p (z s d) -> p z s d", z=2, s=S, d=D)
                    o24 = o2[:].rearrange("p (z s d) -> p z s d", z=2, s=S, d=D)
                    nc.vector.tensor_mul(o24, o4, g_b)
                    nc.vector.tensor_add(o4, o24, b_b)
                nc.scalar.copy(out_f32[:, 0 : S * D], out_bf[:, 0 : S * D])
                nc.scalar.copy(out_f32[:, S * D : ZSD], out_bf[:, S * D : ZSD])
            else:
                nc.scalar.copy(out_f32[:, 0 : S * D], memcat[:, 0 : S * D])
                nc.scalar.copy(out_f32[:, S * D : ZSD], memcat[:, S * D : ZSD])

            # ---- read = sum_s attn*mem ----
            readc = psmall.tile([P, 2, D], dt.float32, tag="readc")
            if "read" not in sk:
                t2 = pscr2.tile([P, ZSD], dt.bfloat16, tag="t2")
                attn_b = (
                    attn_bf[:]
                    .unsqueeze(1)
                    .unsqueeze(3)
                    .broadcast_to((P, 2, S, D))
                )
                mem4bf = mem_bf[:].rearrange("p (z s d) -> p z s d", z=2, s=S, d=D)
                nc.gpsimd.tensor_mul(
                    t2[:].rearrange("p (z s d) -> p z s d", z=2, s=S, d=D), mem4bf, attn_b
                )
                t24 = t2[:].rearrange("p (z s d) -> p z s d", z=2, s=S, d=D)
                hs = S
                while hs > 1:
                    hs //= 2
                    nc.vector.tensor_add(
                        t24[:, :, 0:hs, :], t24[:, :, 0:hs, :], t24[:, :, hs : 2 * hs, :]
                    )
                nc.vector.tensor_copy(readc[:], t24[:, :, 0, :])
            else:
                nc.vector.memset(readc[:], 0.0)

            # ---- DMA out ----
            nc.sync.dma_start(read_r[bsl], readc[:, 0, :])
            nc.sync.dma_start(read_i[bsl], readc[:, 1, :])
            nc.sync.dma_start(next_r[bsl], out_f32[:, 0 : S * D])
            nc.sync.dma_start(next_i[bsl], out_f32[:, S * D : ZSD])

        nc.sync.dma_start(ent[:], ent_acc[:, ntiles % 2 : ntiles % 2 + 1])

    nc.finalize()
    return nc


def _prep_shards(gw_r, gw_i, mem_r, mem_i, Wg, bg, Wa, ba):
    nshard = gw_r.shape[0] // N_CORES
    wcat = np.concatenate([Wa, Wg], axis=1).astype(np.float32)  # [2D, S+1]
    bcat = np.broadcast_to(
        np.concatenate([ba, bg]).astype(np.float32)[None, :], (P, S + 1)
    ).copy()
    ident = np.eye(P, dtype=np.float32)
    in_maps = []
    for c in range(N_CORES):
        sl = slice(c * nshard, (c + 1) * nshard)
        in_maps.append(
            {
                "mem_r": np.ascontiguousarray(mem_r[sl]),
                "mem_i": np.ascontiguousarray(mem_i[sl]),
                "gw_r": np.ascontiguousarray(gw_r[sl]),
                "gw_i": np.ascontiguousarray(gw_i[sl]),
                "wcat": wcat,
                "bcat": bcat,
                "ident": ident,
            }
        )
    return in_maps


def kernel(gw_r, gw_i, mem_r, mem_i, Wg, bg, Wa, ba, g_r, b_r, g_i, b_i, _trace=False):
    gw_r = np.asarray(gw_r, dtype=np.float32)
    gw_i = np.asarray(gw_i, dtype=np.float32)
    mem_r = np.asarray(mem_r, dtype=np.float32)
    mem_i = np.asarray(mem_i, dtype=np.float32)
    Wg = np.asarray(Wg, dtype=np.float32)
    bg = np.asarray(bg, dtype=np.float32)
    Wa = np.asarray(Wa, dtype=np.float32)
    ba = np.asarray(ba, dtype=np.float32)
    g_r = np.asarray(g_r, dtype=np.float32)
    b_r = np.asarray(b_r, dtype=np.float32)
    g_i = np.asarray(g_i, dtype=np.float32)
    b_i = np.asarray(b_i, dtype=np.float32)

    from concourse.bass_utils import run_bass_kernel_spmd

    nb = gw_r.shape[0]
    nshard = nb // N_CORES
    apply_affine = not (
        np.all(g_r == 1.0) and np.all(g_i == 1.0) and np.all(b_r == 0.0) and np.all(b_i == 0.0)
    )
    key = (nshard, apply_affine)
    if key not in _BUILD_CACHE:
        _BUILD_CACHE[key] = build_nc(nshard, apply_affine)
    nc = _BUILD_CACHE[key]

    in_maps = _prep_shards(gw_r, gw_i, mem_r, mem_i, Wg, bg, Wa, ba)
    if apply_affine:
        grep = np.broadcast_to(
            np.concatenate([g_r, g_i]).astype(np.float32)[None, :], (P, 2 * D)
        ).copy()
        brep = np.broadcast_to(
            np.concatenate([b_r, b_i]).astype(np.float32)[None, :], (P, 2 * D)
        ).copy()
        for m in in_maps:
            m["grep"] = grep
            m["brep"] = brep

    res = run_bass_kernel_spmd(nc, in_maps, core_ids=list(range(N_CORES)), trace=False)
    outs = res.results

    next_r_o = np.concatenate([outs[c]["next_r"] for c in range(N_CORES)], axis=0)
    next_i_o = np.concatenate([outs[c]["next_i"] for c in range(N_CORES)], axis=0)
    read_r_o = np.concatenate([outs[c]["read_r"] for c in range(N_CORES)], axis=0)
    read_i_o = np.concatenate([outs[c]["read_i"] for c in range(N_CORES)], axis=0)
    ent_total = sum(float(outs[c]["ent"].sum()) for c in range(N_CORES))
    slot_entropy = np.float32(ent_total / nb)

    return (read_r_o, read_i_o, next_r_o, next_i_o, slot_entropy)


# revision 33
# speedup vs baseline: 3.7482x; 1.0179x over previous
import sys

import numpy as np

sys.path.insert(0, "/opt/trn_rl_repo")

B, S, D, K = 16384, 32, 64, 3
N_CORES = 8
P = 128  # partitions per tile
LN_EPS = 1e-6
ENT_EPS = 1e-10
SUM_EPS = 1e-6

_BUILD_CACHE = {}


def build_nc(nshard, apply_affine=False, use_custom_dve=False, skip=(), repeat=1):
    """Build the single-core Bass/Tile graph for `nshard` samples.

    Layout: partition = sample (128 per tile), free = (z, s, d) with z = re/im.
    The LN of the written memory uses scale invariance:
        LN((1-eu)*mem + eu*gw) = LN(mem + f*gw),  f = eu/(1-eu)
    with the epsilon corrected by (1+f)^2 per (sample, slot).

    `skip` is a set of stage names ("gate","sim","w","stats","apply","read")
    used for performance bisection; skipped stages are replaced by cheap
    constants so the rest of the pipeline still runs.
    """
    from contextlib import ExitStack

    import concourse.bass as bass
    import concourse.mybir as mybir
    from concourse import bacc, tile

    dt = mybir.dt
    AF = mybir.ActivationFunctionType
    ALU = mybir.AluOpType
    AX = mybir.AxisListType

    sk = set(skip)
    ntiles = nshard // P
    ZSD = 2 * S * D  # 4096
    ZD = 2 * D  # 128

    nc = bacc.Bacc()

    mem_r = nc.declare_dram_parameter("mem_r", [nshard, S, D], dt.float32, isOutput=False)
    mem_i = nc.declare_dram_parameter("mem_i", [nshard, S, D], dt.float32, isOutput=False)
    gw_r = nc.declare_dram_parameter("gw_r", [nshard, D], dt.float32, isOutput=False)
    gw_i = nc.declare_dram_parameter("gw_i", [nshard, D], dt.float32, isOutput=False)
    wcat = nc.declare_dram_parameter("wcat", [ZD, S + 1], dt.float32, isOutput=False)
    bcat = nc.declare_dram_parameter("bcat", [P, S + 1], dt.float32, isOutput=False)
    ident = nc.declare_dram_parameter("ident", [P, P], dt.float32, isOutput=False)
    if apply_affine:
        grep = nc.declare_dram_parameter("grep", [P, ZD], dt.float32, isOutput=False)
        brep = nc.declare_dram_parameter("brep", [P, ZD], dt.float32, isOutput=False)

    next_r = nc.declare_dram_parameter("next_r", [nshard, S, D], dt.float32, isOutput=True)
    next_i = nc.declare_dram_parameter("next_i", [nshard, S, D], dt.float32, isOutput=True)
    read_r = nc.declare_dram_parameter("read_r", [nshard, D], dt.float32, isOutput=True)
    read_i = nc.declare_dram_parameter("read_i", [nshard, D], dt.float32, isOutput=True)
    ent = nc.declare_dram_parameter("ent", [P, 1], dt.float32, isOutput=True)

    with tile.TileContext(nc) as tc, ExitStack() as ctx:
        const = ctx.enter_context(tc.tile_pool(name="const", bufs=1))
        pmem = ctx.enter_context(tc.tile_pool(name="pmem", bufs=2))
        pmembf = ctx.enter_context(tc.tile_pool(name="pmembf", bufs=2))
        pscr2 = ctx.enter_context(tc.tile_pool(name="pscr2", bufs=3))
        pw = ctx.enter_context(tc.tile_pool(name="pw", bufs=2))
        pnext = ctx.enter_context(tc.tile_pool(name="pnext", bufs=2))
        pof = ctx.enter_context(tc.tile_pool(name="pof", bufs=3))
        psmall = ctx.enter_context(tc.tile_pool(name="psmall", bufs=3))
        ppsum = ctx.enter_context(
            tc.tile_pool(name="ppsum", bufs=2, space=bass.MemorySpace.PSUM)
        )

        # ---- constants (loaded once) ----
        wcat_t = const.tile([ZD, S + 1], dt.float32)
        nc.sync.dma_start(wcat_t[:], wcat[:])
        bcat_t = const.tile([P, S + 1], dt.float32)
        nc.sync.dma_start(bcat_t[:], bcat[:])
        ident_t = const.tile([P, P], dt.float32)
        nc.sync.dma_start(ident_t[:], ident[:])
        if apply_affine:
            grep_t = const.tile([P, ZD], dt.float32)
            nc.sync.dma_start(grep_t[:], grep[:])
            brep_t = const.tile([P, ZD], dt.float32)
            nc.sync.dma_start(brep_t[:], brep[:])
        ent_acc = const.tile([P, 2], dt.float32)
        nc.vector.memset(ent_acc[:], 0.0)
        entb = const.tile([P, 1], dt.float32)
        nc.vector.memset(entb[:], ENT_EPS)

        for i in range(ntiles * repeat):
            i = i % ntiles
            b0 = i * P
            bsl = slice(b0, b0 + P)

            # ---- DMA in ----
            memcat = pmem.tile([P, ZSD], dt.float32, tag="memcat")
            nc.sync.dma_start(memcat[:, 0 : S * D], mem_r[bsl])
            nc.sync.dma_start(memcat[:, S * D : ZSD], mem_i[bsl])
            gwcat = psmall.tile([P, ZD], dt.float32, tag="gwcat")
            nc.sync.dma_start(gwcat[:, 0:D], gw_r[bsl])
            nc.sync.dma_start(gwcat[:, D:ZD], gw_i[bsl])

            mem4 = memcat[:].rearrange("p (z s d) -> p z s d", z=2, s=S, d=D)
            gw_bz = (
                gwcat[:]
                .rearrange("p (z d) -> p z d", z=2)
                .unsqueeze(2)
                .broadcast_to((P, 2, S, D))
            )

            # bf16 staging copies on ACT (each waits on only one DMA sem)
            mem_bf = pmembf.tile([P, ZSD], dt.bfloat16, tag="membf")
            nc.scalar.copy(mem_bf[:, 0 : S * D], memcat[:, 0 : S * D])
            nc.scalar.copy(mem_bf[:, S * D : ZSD], memcat[:, S * D : ZSD])

            # ---- gate/address nets (PE) + ww softmax + top-k -> f ----
            f_t = psmall.tile([P, S], dt.float32, tag="f")
            if "gate" not in sk:
                gws = psmall.tile([P, ZD], dt.float32, tag="gws")
                nc.scalar.copy(gws[:, 0:D], gwcat[:, 0:D])
                nc.scalar.copy(gws[:, D:ZD], gwcat[:, D:ZD])
                gwT_ps = ppsum.tile([P, P], dt.float32, tag="gwT")
                nc.tensor.transpose(gwT_ps[:], gws[:], ident_t[:])
                flatT = psmall.tile([P, P], dt.float32, tag="flatT")
                nc.scalar.copy(flatT[:], gwT_ps[:])
                lg_ps = ppsum.tile([P, S + 1], dt.float32, tag="lg")
                nc.tensor.matmul(lg_ps[:], flatT[:], wcat_t[:], start=True, stop=True)
                lg = psmall.tile([P, S + 1], dt.float32, tag="lg_sb")
                nc.vector.tensor_add(lg[:], lg_ps[:], bcat_t[:])

                eww = psmall.tile([P, S], dt.float32, tag="eww")
                nc.scalar.activation(eww[:], lg[:, 0:S], AF.Exp)
                zw = psmall.tile([P, 8], dt.float32, tag="zw")
                nc.vector.tensor_reduce(zw[:, 0:1], eww[:], axis=AX.X, op=ALU.add)
                nc.vector.reciprocal(zw[:, 1:2], zw[:, 0:1])
                ww = psmall.tile([P, S], dt.float32, tag="ww")
                nc.vector.tensor_scalar(
                    out=ww[:], in0=eww[:], scalar1=zw[:, 1:2], scalar2=None, op0=ALU.mult
                )
                # gate = sigmoid(lg[:, S]) = 1/(1+exp(-x))
                nc.scalar.activation(zw[:, 2:3], lg[:, S : S + 1], AF.Exp, scale=-1.0)
                nc.vector.tensor_scalar(
                    out=zw[:, 3:4], in0=zw[:, 2:3], scalar1=1.0, scalar2=None, op0=ALU.add
                )
                nc.vector.reciprocal(zw[:, 4:5], zw[:, 3:4])  # gate

                # entropy: ent += -sum ww*ln(ww+eps)
                lw = psmall.tile([P, S], dt.float32, tag="lw")
                nc.scalar.activation(lw[:], ww[:], AF.Ln, bias=entb[:])
                entj = psmall.tile([P, S], dt.float32, tag="entj")
                if use_custom_dve:
                    nc.vector.tensor_tensor_reduce(
                        out=entj[:],
                        in0=ww[:],
                        in1=lw[:],
                        scale=-1.0,
                        scalar=ent_acc[:, i % 2 : i % 2 + 1],
                        op0=ALU.mult,
                        op1=ALU.add,
                        accum_out=ent_acc[:, (i + 1) % 2 : (i + 1) % 2 + 1],
                    )
                else:
                    nc.vector.tensor_mul(entj[:], ww[:], lw[:])
                    nc.vector.tensor_reduce(zw[:, 7:8], entj[:], axis=AX.X, op=ALU.add)
                    nc.vector.scalar_tensor_tensor(
                        out=ent_acc[:, (i + 1) % 2 : (i + 1) % 2 + 1],
                        in0=zw[:, 7:8],
                        scalar=-1.0,
                        in1=ent_acc[:, i % 2 : i % 2 + 1],
                        op0=ALU.mult,
                        op1=ALU.add,
                    )

                # top-3 threshold -> mask -> sparse -> eu -> f = eu/(1-eu)
                mask = psmall.tile([P, S], dt.float32, tag="mask")
                if use_custom_dve:
                    m8 = psmall.tile([P, 8], dt.float32, tag="m8")
                    nc.vector.max(m8[:], ww[:])
                    nc.vector.tensor_scalar(
                        out=mask[:], in0=ww[:], scalar1=m8[:, K - 1 : K],
                        scalar2=None, op0=ALU.is_ge,
                    )
                else:
                    wtmp = psmall.tile([P, S], dt.float32, tag="wtmp")
                    mk = psmall.tile([P, 4], dt.float32, tag="mk")
                    eqm = psmall.tile([P, S], dt.float32, tag="eqm")
                    nc.vector.tensor_reduce(mk[:, 0:1], ww[:], axis=AX.X, op=ALU.max)
                    nc.vector.tensor_scalar(
                        out=eqm[:], in0=ww[:], scalar1=mk[:, 0:1], scalar2=None, op0=ALU.is_ge
                    )
                    nc.vector.scalar_tensor_tensor(
                        out=wtmp[:], in0=eqm[:], scalar=-2.0, in1=ww[:], op0=ALU.mult, op1=ALU.add
                    )
                    nc.vector.tensor_reduce(mk[:, 1:2], wtmp[:], axis=AX.X, op=ALU.max)
                    nc.vector.tensor_scalar(
                        out=eqm[:], in0=wtmp[:], scalar1=mk[:, 1:2], scalar2=None, op0=ALU.is_ge
                    )
                    nc.vector.scalar_tensor_tensor(
                        out=wtmp[:], in0=eqm[:], scalar=-2.0, in1=wtmp[:], op0=ALU.mult, op1=ALU.add
                    )
                    nc.vector.tensor_reduce(mk[:, 2:3], wtmp[:], axis=AX.X, op=ALU.max)
                    nc.vector.tensor_scalar(
                        out=mask[:], in0=ww[:], scalar1=mk[:, 2:3], scalar2=None, op0=ALU.is_ge
                    )
                spv = psmall.tile([P, S], dt.float32, tag="spv")
                if use_custom_dve:
                    nc.vector.tensor_tensor_reduce(
                        out=spv[:],
                        in0=ww[:],
                        in1=mask[:],
                        scale=1.0,
                        scalar=float(SUM_EPS),
                        op0=ALU.mult,
                        op1=ALU.add,
                        accum_out=zw[:, 5:6],
                    )
                else:
                    nc.vector.tensor_mul(spv[:], ww[:], mask[:])
                    nc.vector.tensor_reduce(zw[:, 6:7], spv[:], axis=AX.X, op=ALU.add)
                    nc.vector.tensor_scalar(
                        out=zw[:, 5:6], in0=zw[:, 6:7], scalar1=float(SUM_EPS),
                        scalar2=None, op0=ALU.add,
                    )
                nc.vector.reciprocal(zw[:, 6:7], zw[:, 5:6])
                eu = psmall.tile([P, S], dt.float32, tag="eu")
                nc.vector.tensor_scalar(
                    out=eu[:],
                    in0=spv[:],
                    scalar1=zw[:, 6:7],
                    scalar2=zw[:, 4:5],
                    op0=ALU.mult,
                    op1=ALU.mult,
                )
                ome = psmall.tile([P, S], dt.float32, tag="ome")
                nc.vector.tensor_scalar(
                    out=ome[:], in0=eu[:], scalar1=-1.0, scalar2=1.0, op0=ALU.mult, op1=ALU.add
                )
                rome = psmall.tile([P, S], dt.float32, tag="rome")
                nc.vector.reciprocal(rome[:], ome[:])
                nc.vector.tensor_mul(f_t[:], eu[:], rome[:])
            else:
                nc.vector.memset(f_t[:], 0.1)

            # ---- similarity + attn softmax ----
            attn_bf = psmall.tile([P, S], dt.bfloat16, tag="attnbf")
            if "sim" not in sk:
                # split r/i so each TT waits on only one mem DMA (sync-wait cap)
                t1 = pof.tile([P, ZSD], dt.float32, tag="t1")
                t14 = t1[:].rearrange("p (z s d) -> p z s d", z=2, s=S, d=D)
                nc.vector.tensor_mul(t14[:, 0], mem4[:, 0], gw_bz[:, 0])
                nc.vector.tensor_mul(t14[:, 1], mem4[:, 1], gw_bz[:, 1])
                simzs = psmall.tile([P, 2 * S], dt.float32, tag="simzs")
                nc.vector.tensor_reduce(
                    simzs[:],
                    t1[:].rearrange("p (g d) -> p g d", g=2 * S, d=D),
                    axis=AX.X,
                    op=ALU.add,
                )
                simt = psmall.tile([P, S], dt.float32, tag="sim")
                nc.vector.tensor_add(simt[:], simzs[:, 0:S], simzs[:, S : 2 * S])
                # softmax without max-subtract; |sim| <~ 60 is exp-safe
                es = psmall.tile([P, S], dt.float32, tag="es")
                nc.scalar.activation(es[:], simt[:], AF.Exp)
                zr = psmall.tile([P, 4], dt.float32, tag="zr")
                nc.vector.tensor_reduce(zr[:, 0:1], es[:], axis=AX.X, op=ALU.add)
                nc.vector.reciprocal(zr[:, 1:2], zr[:, 0:1])
                nc.vector.tensor_scalar(
                    out=attn_bf[:], in0=es[:], scalar1=zr[:, 1:2], scalar2=None, op0=ALU.mult
                )
            else:
                nc.vector.memset(attn_bf[:], 1.0 / S)

            # ---- w = f*gw; next' = mem + w ----
            if "w" not in sk:
                gw_bf = psmall.tile([P, ZD], dt.bfloat16, tag="gwbf")
                nc.scalar.copy(gw_bf[:, 0:D], gwcat[:, 0:D])
                nc.scalar.copy(gw_bf[:, D:ZD], gwcat[:, D:ZD])
                w_bf = pw.tile([P, ZSD], dt.bfloat16, tag="w")
                w4 = w_bf[:].rearrange("p (z s d) -> p z s d", z=2, s=S, d=D)
                gw3 = gw_bf[:].rearrange("p (z d) -> p z d", z=2)
                for s in range(S):
                    nc.scalar.activation(
                        w4[:, :, s, :], gw3, AF.Copy, scale=f_t[:, s : s + 1]
                    )
                next_bf = pnext.tile([P, ZSD], dt.bfloat16, tag="next")
                nc.vector.tensor_add(next_bf[:], mem_bf[:], w_bf[:])
            else:
                next_bf = mem_bf

            # ---- LN stats of next' ----
            rs = psmall.tile([P, 2 * S], dt.float32, tag="rs")
            cc = psmall.tile([P, 2 * S], dt.float32, tag="cc")
            if "stats" not in sk:
                sq_bf = pscr2.tile([P, ZSD], dt.bfloat16, tag="sqbf")
                nc.scalar.square(sq_bf[:], next_bf[:])
                sn = psmall.tile([P, 2 * S], dt.float32, tag="sn")
                snt = pscr2.tile([P, 2 * S, D // 2], dt.bfloat16, tag="t2")
                nx4 = next_bf[:].rearrange("p (g d) -> p g d", g=2 * S, d=D)
                nc.vector.tensor_add(snt[:], nx4[:, :, 0 : D // 2], nx4[:, :, D // 2 : D])
                hd = D // 2
                while hd > 2:
                    hd //= 2
                    nc.vector.tensor_add(
                        snt[:, :, 0:hd], snt[:, :, 0:hd], snt[:, :, hd : 2 * hd]
                    )
                nc.vector.tensor_add(sn[:], snt[:, :, 0], snt[:, :, 1])
                sq = psmall.tile([P, 2 * S], dt.float32, tag="sqs")
                sq4 = sq_bf[:].rearrange("p (g d) -> p g d", g=2 * S, d=D)
                hd = D
                while hd > 2:
                    hd //= 2
                    nc.vector.tensor_add(
                        sq4[:, :, 0:hd], sq4[:, :, 0:hd], sq4[:, :, hd : 2 * hd]
                    )
                nc.vector.tensor_add(sq[:], sq4[:, :, 0], sq4[:, :, 1])
                mmean = psmall.tile([P, 2 * S], dt.float32, tag="mmean")
                nc.vector.tensor_scalar(
                    out=mmean[:], in0=sn[:], scalar1=1.0 / D, scalar2=None, op0=ALU.mult
                )
                msq = psmall.tile([P, 2 * S], dt.float32, tag="msq")
                nc.scalar.square(msq[:], mmean[:])
                varr = psmall.tile([P, 2 * S], dt.float32, tag="varr")
                nc.vector.scalar_tensor_tensor(
                    out=varr[:], in0=sq[:], scalar=1.0 / D, in1=msq[:],
                    op0=ALU.mult, op1=ALU.subtract,
                )
                # eps correction: varf = varr + LN_EPS*(1+f)^2
                opf2 = psmall.tile([P, S], dt.float32, tag="opf2")
                nc.scalar.activation(opf2[:], f_t[:], AF.Square, bias=1.0)
                varf = psmall.tile([P, 2 * S], dt.float32, tag="varf")
                nc.vector.scalar_tensor_tensor(
                    out=varf[:].rearrange("p (z s) -> p z s", z=2),
                    in0=opf2[:].unsqueeze(1).broadcast_to((P, 2, S)),
                    scalar=LN_EPS,
                    in1=varr[:].rearrange("p (z s) -> p z s", z=2),
                    op0=ALU.mult,
                    op1=ALU.add,
                )
                # rs = 1/sqrt(varf) via exp(-0.5*ln(varf))
                lv = psmall.tile([P, 2 * S], dt.float32, tag="lv")
                nc.scalar.activation(lv[:], varf[:], AF.Ln)
                nc.scalar.activation(rs[:], lv[:], AF.Exp, scale=-0.5)
                nc.vector.scalar_tensor_tensor(
                    out=cc[:], in0=mmean[:], scalar=-1.0, in1=rs[:], op0=ALU.mult, op1=ALU.mult
                )
            else:
                nc.vector.memset(rs[:], 1.0)
                nc.vector.memset(cc[:], 0.0)

            # ---- apply LN: out = next'*rs + cc, per (z,s) group ----
            if "apply" not in sk:
                out_bf = pw.tile([P, ZSD], dt.bfloat16, tag="w")
                for g in range(2 * S):
                    nc.vector.tensor_scalar(
                        out=out_bf[:, g * D : (g + 1) * D],
                        in0=next_bf[:, g * D : (g + 1) * D],
                        scalar1=rs[:, g : g + 1],
                        scalar2=cc[:, g : g + 1],
                        op0=ALU.mult,
                        op1=ALU.add,
                    )
                if apply_affine:
                    o2 = pscr2.tile([P, ZSD], dt.bfloat16, tag="o2")
                    g_b = (
                        grep_t[:]
                        .rearrange("p (z d) -> p z d", z=2)
                        .unsqueeze(2)
                        .broadcast_to((P, 2, S, D))
                    )
                    b_b = (
                        brep_t[:]
                        .rearrange("p (z d) -> p z d", z=2)
                        .unsqueeze(2)
                        .broadcast_to((P, 2, S, D))
                    )
                    o4 = out_bf[:].rearrange("p (z s d) -> p z s d", z=2, s=S, d=D)
                    o24 = o2[:].rearrange("p (z s d) -> p z s d", z=2, s=S, d=D)
                    nc.vector.tensor_mul(o24, o4, g_b)
                    nc.vector.tensor_add(o4, o24, b_b)
                pass
            else:
                out_bf = pw.tile([P, ZSD], dt.bfloat16, tag="w")
                nc.scalar.copy(out_bf[:, 0 : S * D], memcat[:, 0 : S * D])
                nc.scalar.copy(out_bf[:, S * D : ZSD], memcat[:, S * D : ZSD])

            # ---- read = sum_s attn*mem ----
            readc = psmall.tile([P, 2, D], dt.float32, tag="readc")
            if "read" not in sk:
                t2 = pscr2.tile([P, ZSD], dt.bfloat16, tag="t2")
                attn_b = (
                    attn_bf[:]
                    .unsqueeze(1)
                    .unsqueeze(3)
                    .broadcast_to((P, 2, S, D))
                )
                mem4bf = mem_bf[:].rearrange("p (z s d) -> p z s d", z=2, s=S, d=D)
                nc.gpsimd.tensor_mul(
                    t2[:].rearrange("p (z s d) -> p z s d", z=2, s=S, d=D), mem4bf, attn_b
                )
                t24 = t2[:].rearrange("p (z s d) -> p z s d", z=2, s=S, d=D)
                hs = S
                while hs > 1:
                    hs //= 2
                    nc.vector.tensor_add(
                        t24[:, :, 0:hs, :], t24[:, :, 0:hs, :], t24[:, :, hs : 2 * hs, :]
                    )
                nc.vector.tensor_copy(readc[:], t24[:, :, 0, :])
            else:
                nc.vector.memset(readc[:], 0.0)

            # ---- DMA out ----
            nc.sync.dma_start(read_r[bsl], readc[:, 0, :])
            nc.sync.dma_start(read_i[bsl], readc[:, 1, :])
            nc.gpsimd.dma_start(next_r[bsl], out_bf[:, 0 : S * D])
            nc.gpsimd.dma_start(next_i[bsl], out_bf[:, S * D : ZSD])

        nc.sync.dma_start(ent[:], ent_acc[:, ntiles % 2 : ntiles % 2 + 1])

    nc.finalize()
    return nc


def _prep_shards(gw_r, gw_i, mem_r, mem_i, Wg, bg, Wa, ba):
    nshard = gw_r.shape[0] // N_CORES
    wcat = np.concatenate([Wa, Wg], axis=1).astype(np.float32)  # [2D, S+1]
    bcat = np.broadcast_to(
        np.concatenate([ba, bg]).astype(np.float32)[None, :], (P, S + 1)
    ).copy()
    ident = np.eye(P, dtype=np.float32)
    in_maps = []
    for c in range(N_CORES):
        sl = slice(c * nshard, (c + 1) * nshard)
        in_maps.append(
            {
                "mem_r": np.ascontiguousarray(mem_r[sl]),
                "mem_i": np.ascontiguousarray(mem_i[sl]),
                "gw_r": np.ascontiguousarray(gw_r[sl]),
                "gw_i": np.ascontiguousarray(gw_i[sl]),
                "wcat": wcat,
                "bcat": bcat,
                "ident": ident,
            }
        )
    return in_maps


def kernel(gw_r, gw_i, mem_r, mem_i, Wg, bg, Wa, ba, g_r, b_r, g_i, b_i, _trace=False):
    gw_r = np.asarray(gw_r, dtype=np.float32)
    gw_i = np.asarray(gw_i, dtype=np.float32)
    mem_r = np.asarray(mem_r, dtype=np.float32)
    mem_i = np.asarray(mem_i, dtype=np.float32)
    Wg = np.asarray(Wg, dtype=np.float32)
    bg = np.asarray(bg, dtype=np.float32)
    Wa = np.asarray(Wa, dtype=np.float32)
    ba = np.asarray(ba, dtype=np.float32)
    g_r = np.asarray(g_r, dtype=np.float32)
    b_r = np.asarray(b_r, dtype=np.float32)
    g_i = np.asarray(g_i, dtype=np.float32)
    b_i = np.asarray(b_i, dtype=np.float32)

    from concourse.bass_utils import run_bass_kernel_spmd

    nb = gw_r.shape[0]
    nshard = nb // N_CORES
    apply_affine = not (
        np.all(g_r == 1.0) and np.all(g_i == 1.0) and np.all(b_r == 0.0) and np.all(b_i == 0.0)
    )
    key = (nshard, apply_affine)
    if key not in _BUILD_CACHE:
        _BUILD_CACHE[key] = build_nc(nshard, apply_affine)
    nc = _BUILD_CACHE[key]

    in_maps = _prep_shards(gw_r, gw_i, mem_r, mem_i, Wg, bg, Wa, ba)
    if apply_affine:
        grep = np.broadcast_to(
            np.concatenate([g_r, g_i]).astype(np.float32)[None, :], (P, 2 * D)
        ).copy()
        brep = np.broadcast_to(
            np.concatenate([b_r, b_i]).astype(np.float32)[None, :], (P, 2 * D)
        ).copy()
        for m in in_maps:
            m["grep"] = grep
            m["brep"] = brep

    res = run_bass_kernel_spmd(nc, in_maps, core_ids=list(range(N_CORES)), trace=False)
    outs = res.results

    next_r_o = np.concatenate([outs[c]["next_r"] for c in range(N_CORES)], axis=0)
    next_i_o = np.concatenate([outs[c]["next_i"] for c in range(N_CORES)], axis=0)
    read_r_o = np.concatenate([outs[c]["read_r"] for c in range(N_CORES)], axis=0)
    read_i_o = np.concatenate([outs[c]["read_i"] for c in range(N_CORES)], axis=0)
    ent_total = sum(float(outs[c]["ent"].sum()) for c in range(N_CORES))
    slot_entropy = np.float32(ent_total / nb)

    return (read_r_o, read_i_o, next_r_o, next_i_o, slot_entropy)
